# revision 31
# baseline (speedup 1.0000x reference)
"""Trainium2 Bass kernel for nn_AttnBlock (GroupNorm + single-head attention
over 4096 positions + output projection + residual), distributed over 8
NeuronCores.

Sharding: core (4*b + s), b in {0,1} batches, s in {0..3} query-quarters.
Each core:
  - group-norms its batch's full [512, 4096] activation (stats on device:
    per-group sum / sum-of-squares via indicator matmuls on the PE),
  - projects k for ALL 4096 key positions, q/v for its 1024-query quarter,
  - computes scores = qT.T @ k, exp (no max-subtraction: |score| <= ~8),
    row sums Z_i via DVE reduction,
  - folds the output projection in early: MT_i = (v_i.T @ wp.T) * (1/Z_i),
    then y_partial = sum_i MT_i.T @ exp_scores_i  (valid because this
    AttnBlock contracts over the *query* index: out_j = sum_i v_i w[i,j]),
  - returns y_partial [512, 4096] fp32.
Host glue: slice inputs per core, transpose+bf16-cast weights, sum the 4
query-quarter partials per batch, add output bias + residual.
"""

import os
import sys

for _p in ("/opt/trn_rl_repo", "/root/.axon_site/_ro/trn_rl_repo"):
    if _p not in sys.path and os.path.isdir(_p):
        sys.path.insert(0, _p)

import numpy as np
import ml_dtypes

BF = ml_dtypes.bfloat16
F8 = ml_dtypes.float8_e4m3

# Problem dims (hardcoded per spec)
B, C, HH, WW = 2, 512, 64, 64
N = HH * WW            # 4096 key/output positions
NQ = N // 4            # 1024 query positions per core
P = 128                # partitions
CT = C // P            # 4 channel tiles
JCH = 512              # j (key) chunk = max moving free dim
NJ = N // JCH          # 8 chunks
IT = NQ // P           # 8 query i-tiles per core
NSUB = N // 512        # 512-wide stats chunks per row tile
G, GS = 32, 16         # groups, channels per group
NELEM = GS * N         # elements per group
EPS = 1e-6
SCALE = float(C) ** -0.5

_CACHE = {}


def _build_nc(finalize=True):
    import concourse.bacc as bacc
    import concourse.bass as bass
    import concourse.tile as tile
    from concourse import mybir

    f32 = mybir.dt.float32
    bf16 = mybir.dt.bfloat16
    f8 = mybir.dt.float8e4
    AX = mybir.AxisListType
    OP = mybir.AluOpType
    AF = mybir.ActivationFunctionType
    DR = mybir.MatmulPerfMode.DoubleRow

    nc = bacc.Bacc(
        "TRN2",
        target_bir_lowering=False,
        debug=False,
        enable_asserts=False,
        num_devices=8,
    )

    # ---- DRAM I/O ----
    x_d = nc.dram_tensor("x", [C, N], f8, kind="ExternalInput").ap()
    xq_d = nc.dram_tensor("xq", [C, NQ], f8, kind="ExternalInput").ap()
    wqT_d = nc.dram_tensor("wqT", [C, C], bf16, kind="ExternalInput").ap()
    wkT_d = nc.dram_tensor("wkT", [C, C], bf16, kind="ExternalInput").ap()
    wvT_d = nc.dram_tensor("wvT", [C, C], bf16, kind="ExternalInput").ap()
    wpT_d = nc.dram_tensor("wpT", [C, C], bf16, kind="ExternalInput").ap()
    # vecs rows: 0=bq 1=bk 2=bv 3=norm_w 4=norm_b
    vecs_d = nc.dram_tensor("vecs", [5, C], f32, kind="ExternalInput").ap()
    indb_d = nc.dram_tensor("indb", [P, CT, G], f8, kind="ExternalInput").ap()
    indt_d = nc.dram_tensor("indt", [G, CT, P], f32, kind="ExternalInput").ap()
    y_d = nc.dram_tensor("y", [C, N], f32, kind="ExternalOutput").ap()

    x_r = x_d.rearrange("(t p) n -> t p n", p=P)
    xq_r = xq_d.rearrange("(t p) n -> t p n", p=P)
    y_r = y_d.rearrange("(t p) n -> t p n", p=P)
    w_src = {
        "q": wqT_d.rearrange("(t p) o -> p t o", p=P),
        "k": wkT_d.rearrange("(t p) o -> p t o", p=P),
        "v": wvT_d.rearrange("(t p) o -> p t o", p=P),
        "p": wpT_d.rearrange("(t p) o -> p t o", p=P),
    }
    vecs_src = vecs_d.rearrange("v (t p) -> p v t", p=P)

    with tile.TileContext(nc) as tc:
        with tc.tile_pool(name="singles", bufs=1) as singles, tc.tile_pool(
            name="work", bufs=4
        ) as work:
            # ---- persistent tiles ----
            wsb = {
                nm: singles.tile(
                    [P, CT, JCH], bf16, tag=f"w{nm}", name=f"w{nm}"
                )
                for nm in ("q", "k", "v", "p")
            }
            h = [singles.tile([P, N], bf16, tag=f"h{t}", name=f"h{t}") for t in range(CT)]
            hq = [singles.tile([P, NQ], bf16, tag=f"hq{t}", name=f"hq{t}") for t in range(CT)]
            k8 = singles.tile([P, CT, N], f8, tag="k8", name="k8")
            q8 = singles.tile([P, CT, NQ], f8, tag="q8", name="q8")
            v_sb = [singles.tile([P, NQ], bf16, tag=f"v{t}", name=f"v{t}") for t in range(CT)]
            MT_sb = [singles.tile([P, C], bf16, tag=f"mt{i}", name=f"mt{i}") for i in range(IT)]
            vec_sb = singles.tile([P, 5, CT], f32, tag="vecs", name="vecs")
            indb_sb = singles.tile([P, CT, G], f8, tag="indb", name="indb")
            indt_sb = singles.tile([G, CT, P], f32, tag="indt", name="indt")
            eps_sb = singles.tile([G, 1], f32, tag="eps", name="eps")
            gval = singles.tile([G, 2], f32, tag="gval", name="gval")  # mu, rstd
            gtmp = singles.tile([G, 2], f32, tag="gtmp", name="gtmp")
            scale_sb = singles.tile([P, CT], f32, tag="scale", name="scale")
            bias_sb = singles.tile([P, CT], f32, tag="bias", name="bias")
            zrec = singles.tile([P, IT], f32, tag="zrec", name="zrec")

            bq_ap = [vec_sb[:, 0, t : t + 1] for t in range(CT)]
            bk_ap = [vec_sb[:, 1, t : t + 1] for t in range(CT)]
            bv_ap = [vec_sb[:, 2, t : t + 1] for t in range(CT)]
            nw_ap = [vec_sb[:, 3, t : t + 1] for t in range(CT)]
            nb_ap = [vec_sb[:, 4, t : t + 1] for t in range(CT)]

            nc.vector.memset(eps_sb, EPS)

            # ================= Phase A: load x, groupnorm stats, apply =======
            with tc.tile_pool(name="xpool", bufs=1) as xpool, tc.tile_pool(
                name="x2pool", bufs=6
            ) as x2pool, tc.tile_pool(name="gps", bufs=1, space="PSUM") as gps:
                x_sb = [xpool.tile([P, N], f8, tag=f"x{t}", name=f"x{t}") for t in range(CT)]
                xq_sb = [xpool.tile([P, NQ], f8, tag=f"xq{t}", name=f"xq{t}") for t in range(CT)]

                # x (and xq) on the sync HWDGE queue; multi-dim-AP loads on
                # the scalar queue
                for t in range(CT):
                    for hh in range(2):
                        cs = slice(hh * (N // 2), (hh + 1) * (N // 2))
                        nc.sync.dma_start(out=x_sb[t][:, cs], in_=x_r[t][:, cs])
                for t in range(CT):
                    nc.sync.dma_start(out=xq_sb[t], in_=xq_r[t])
                nc.scalar.dma_start(out=indb_sb, in_=indb_d)
                nc.scalar.dma_start(out=indt_sb, in_=indt_d)
                nc.scalar.dma_start(out=vec_sb, in_=vecs_src)
                for nm in ("q", "k", "v", "p"):
                    nc.scalar.dma_start(out=wsb[nm], in_=w_src[nm])

                # group sums + sums of squares via indicator matmuls
                ps_sum = gps.tile([G, JCH], f32, tag="pssum", name="pssum")
                ps_sq = gps.tile([G, JCH], f32, tag="pssq", name="pssq")
                nmm = CT * NSUB
                # all Sum(x) matmuls (chunk-ordered so they chase the DMAs)
                for t in range(CT):
                    for s in range(NSUB):
                        cs = slice(s * 512, (s + 1) * 512)
                        i_mm = t * NSUB + s
                        nc.tensor.matmul(
                            ps_sum,
                            indb_sb[:, t, :],
                            x_sb[t][:, cs],
                            start=(i_mm == 0),
                            stop=(i_mm == nmm - 1),
                        )
                # squares on DVE (bf16 2x mode), then Sum(x^2) matmuls
                for t in range(CT):
                    for s in range(NSUB):
                        cs = slice(s * 512, (s + 1) * 512)
                        i_mm = t * NSUB + s
                        x2c = x2pool.tile([P, JCH], f8, tag="x2", name="x2")
                        nc.vector.tensor_mul(x2c, x_sb[t][:, cs], x_sb[t][:, cs])
                        nc.tensor.matmul(
                            ps_sq,
                            indb_sb[:, t, :],
                            x2c,
                            start=(i_mm == 0),
                            stop=(i_mm == nmm - 1),
                        )

                # reduce to per-group scalars and build (mu, rstd)
                nc.vector.reduce_sum(out=gtmp[:, 0:1], in_=ps_sum, axis=AX.X)
                nc.vector.reduce_sum(out=gtmp[:, 1:2], in_=ps_sq, axis=AX.X)
                nc.vector.tensor_scalar_mul(gval[:, 0:1], gtmp[:, 0:1], 1.0 / NELEM)
                nc.vector.tensor_scalar_mul(gtmp[:, 1:2], gtmp[:, 1:2], 1.0 / NELEM)
                nc.vector.tensor_mul(gtmp[:, 0:1], gval[:, 0:1], gval[:, 0:1])
                nc.vector.tensor_sub(gtmp[:, 0:1], gtmp[:, 1:2], gtmp[:, 0:1])  # var
                nc.scalar.activation(
                    out=gtmp[:, 0:1],
                    in_=gtmp[:, 0:1],
                    func=AF.Sqrt,
                    bias=eps_sb,
                    scale=1.0,
                )
                nc.vector.reciprocal(out=gval[:, 1:2], in_=gtmp[:, 0:1])  # rstd

                # broadcast group (mu, rstd) to channels via transposed
                # indicator matmuls, then per-channel scale/bias
                for t in range(CT):
                    pbc = gps.tile([P, 2], f32, tag="pbc", name="pbc", bufs=4)
                    nc.tensor.matmul(
                        pbc, indt_sb[:, t, :], gval, start=True, stop=True
                    )
                    nc.vector.tensor_mul(
                        scale_sb[:, t : t + 1], nw_ap[t], pbc[:, 1:2]
                    )
                    nc.vector.tensor_mul(
                        bias_sb[:, t : t + 1], pbc[:, 0:1], scale_sb[:, t : t + 1]
                    )
                    nc.vector.tensor_sub(
                        bias_sb[:, t : t + 1], nb_ap[t], bias_sb[:, t : t + 1]
                    )

                # apply: h = scale*x + bias (bf16), DVE/ACT alternating
                def apply_chunk(dst, src, t, cs, use_act):
                    if use_act:
                        nc.scalar.activation(
                            out=dst[:, cs],
                            in_=src[:, cs],
                            func=AF.Identity,
                            bias=bias_sb[:, t : t + 1],
                            scale=scale_sb[:, t : t + 1],
                        )
                    else:
                        nc.vector.tensor_scalar(
                            out=dst[:, cs],
                            in0=src[:, cs],
                            scalar1=scale_sb[:, t : t + 1],
                            scalar2=bias_sb[:, t : t + 1],
                            op0=OP.mult,
                            op1=OP.add,
                        )

                for s in range(NSUB):
                    cs = slice(s * 512, (s + 1) * 512)
                    for t in range(CT):
                        apply_chunk(h[t], x_sb[t], t, cs, use_act=(t % 2 == 1))
                for s in range(NQ // 512):
                    cs = slice(s * 512, (s + 1) * 512)
                    for t in range(CT):
                        apply_chunk(hq[t], xq_sb[t], t, cs, use_act=(t % 2 == 1))

            # ================= Phase B: projections, attention ===============
            with tc.tile_pool(name="wpool", bufs=1) as wpool, tc.tile_pool(
                name="mm1", bufs=2, space="PSUM"
            ) as mm1, tc.tile_pool(
                name="mm2", bufs=3, space="PSUM"
            ) as mm2, tc.tile_pool(name="ypool", bufs=4) as ypool:
                w_sb = [wpool.tile([P, N], bf16, tag=f"ws{i}", name=f"ws{i}") for i in range(IT)]

                # k projection (full N; psum->sbuf+bias on ACT),
                # q/v projections (quarter; on DVE)
                for co in range(CT):
                    osl = slice(co * P, (co + 1) * P)
                    for ch in range(NJ):
                        cs = slice(ch * JCH, (ch + 1) * JCH)
                        ps = mm1.tile([P, JCH], f32, tag="mm", name="mm")
                        for ti in range(CT):
                            nc.tensor.matmul(
                                ps,
                                wsb["k"][:, ti, osl],
                                h[ti][:, cs],
                                start=(ti == 0),
                                stop=(ti == CT - 1),
                            )
                        if ch % 3 == 2:
                            nc.scalar.activation(
                                out=k8[:, co, cs],
                                in_=ps,
                                func=AF.Identity,
                                bias=bk_ap[co],
                                scale=1.0,
                            )
                        else:
                            nc.vector.tensor_scalar_add(
                                out=k8[:, co, cs], in0=ps, scalar1=bk_ap[co]
                            )
                    for ch in range(NQ // JCH):
                        cs = slice(ch * JCH, (ch + 1) * JCH)
                        psq = mm1.tile([P, JCH], f32, tag="mm", name="mm")
                        for ti in range(CT):
                            nc.tensor.matmul(
                                psq,
                                wsb["q"][:, ti, osl],
                                hq[ti][:, cs],
                                start=(ti == 0),
                                stop=(ti == CT - 1),
                            )
                        nc.vector.tensor_scalar_add(
                            out=q8[:, co, cs], in0=psq, scalar1=bq_ap[co]
                        )
                        psv = mm1.tile([P, JCH], f32, tag="mm", name="mm")
                        for ti in range(CT):
                            nc.tensor.matmul(
                                psv,
                                wsb["v"][:, ti, osl],
                                hq[ti][:, cs],
                                start=(ti == 0),
                                stop=(ti == CT - 1),
                            )
                        nc.vector.tensor_scalar_add(
                            out=v_sb[co][:, cs], in0=psv, scalar1=bv_ap[co]
                        )

                # QK^T + exp + row sums + MT per query i-tile
                for i in range(IT):
                    isl = slice(i * P, (i + 1) * P)
                    zs = work.tile([P, 4], f32, tag="zs", name="zs")
                    for c2 in range(4):  # 1024-wide double chunks
                        ps2 = mm2.tile([P, 2, JCH], f32, tag="qk", name="qk")
                        for hh in range(2):
                            cs = slice(
                                (c2 * 2 + hh) * JCH, (c2 * 2 + hh + 1) * JCH
                            )
                            for tp in range(2):  # fp8 double-row: 2 c-tile pairs
                                nc.tensor.matmul(
                                    ps2[:, hh, :],
                                    q8[:, 2 * tp : 2 * tp + 2, isl],
                                    k8[:, 2 * tp : 2 * tp + 2, cs],
                                    start=(tp == 0),
                                    stop=(tp == 1),
                                    perf_mode=DR,
                                )
                        wview = w_sb[i][
                            :, c2 * 2 * JCH : (c2 + 1) * 2 * JCH
                        ].rearrange("p (a b) -> p a b", a=2)
                        nc.scalar.activation(
                            out=wview,
                            in_=ps2,
                            func=AF.Exp,
                            bias=0.0,
                            scale=SCALE,
                            accum_out=zs[:, c2 : c2 + 1],
                        )
                    zt = work.tile([P, 1], f32, tag="zt", name="zt")
                    nc.vector.reduce_sum(out=zt, in_=zs, axis=AX.X)
                    nc.vector.reciprocal(out=zrec[:, i : i + 1], in_=zt)
                    # MT_i = (v_i-tile.T @ wpT) * (1/Z_i)   [128 i, 512 o]
                    pm = mm1.tile([P, C], f32, tag="mm", name="mm")
                    for cc in range(CT):
                        nc.tensor.matmul(
                            pm,
                            v_sb[cc][:, isl],
                            wsb["p"][:, cc, :],
                            start=(cc == 0),
                            stop=(cc == CT - 1),
                        )
                    nc.vector.tensor_scalar_mul(
                        out=MT_sb[i], in0=pm, scalar1=zrec[:, i : i + 1]
                    )

                # y = sum_i MT_i.T @ w_i    [512 o, 4096 j]
                for oo in range(CT):
                    osl = slice(oo * P, (oo + 1) * P)
                    for ch in range(NJ):
                        cs = slice(ch * JCH, (ch + 1) * JCH)
                        ps = mm1.tile([P, JCH], f32, tag="mm", name="mm")
                        for i in range(IT):
                            nc.tensor.matmul(
                                ps,
                                MT_sb[i][:, osl],
                                w_sb[i][:, cs],
                                start=(i == 0),
                                stop=(i == IT - 1),
                            )
                        yc = ypool.tile([P, JCH], f32, tag="yc", name="yc")
                        if ch % 2 == 0:
                            nc.scalar.copy(out=yc, in_=ps)
                        else:
                            nc.vector.tensor_copy(out=yc, in_=ps)
                        nc.sync.dma_start(out=y_r[oo][:, cs], in_=yc)

    if finalize:
        nc.finalize()
    return nc


def _get_nc():
    if "nc" not in _CACHE:
        _CACHE["nc"] = _build_nc()
    return _CACHE["nc"]


def _make_inds():
    indb = np.zeros((P, CT, G), F8)
    indt = np.zeros((G, CT, P), np.float32)
    for t in range(CT):
        for c in range(P):
            g = 8 * t + c // GS
            indb[c, t, g] = 1.0
            indt[g, t, c] = 1.0
    return indb, indt


def prepare_in_maps(inputs):
    x = np.asarray(inputs["x"], np.float32)
    xr = x.reshape(B, C, N)
    x_bf = [np.ascontiguousarray(xr[b]).astype(F8) for b in range(B)]
    wT = {
        nm: np.ascontiguousarray(np.asarray(inputs[nm], np.float32).T).astype(BF)
        for nm in ("wq", "wk", "wv", "wp")
    }
    vecs = np.stack(
        [
            np.asarray(inputs["bq"], np.float32),
            np.asarray(inputs["bk"], np.float32),
            np.asarray(inputs["bv"], np.float32),
            np.asarray(inputs["norm_w"], np.float32),
            np.asarray(inputs["norm_b"], np.float32),
        ]
    )
    indb, indt = _make_inds()
    shared = {
        "wqT": wT["wq"],
        "wkT": wT["wk"],
        "wvT": wT["wv"],
        "wpT": wT["wp"],
        "vecs": vecs,
        "indb": indb,
        "indt": indt,
    }
    in_maps = []
    for b in range(B):
        for s in range(4):
            m = dict(shared)
            m["x"] = x_bf[b]
            m["xq"] = np.ascontiguousarray(x_bf[b][:, s * NQ : (s + 1) * NQ])
            in_maps.append(m)
    return in_maps


def kernel(**inputs):
    from concourse.bass_utils import run_bass_kernel_spmd

    nc = _get_nc()
    in_maps = prepare_in_maps(inputs)
    res = run_bass_kernel_spmd(nc, in_maps, core_ids=list(range(8)))
    ys = [np.asarray(r["y"], np.float32) for r in res.results]

    x = np.asarray(inputs["x"], np.float32)
    xr = x.reshape(B, C, N)
    bp = np.asarray(inputs["bp"], np.float32).reshape(C, 1)
    out = np.empty((B, C, N), np.float32)
    for b in range(B):
        acc = ys[4 * b] + ys[4 * b + 1] + ys[4 * b + 2] + ys[4 * b + 3]
        out[b] = acc + bp + xr[b]
    return out.reshape(B, C, HH, WW)


if __name__ == "__main__":
    rng = np.random.default_rng(0)
    fake = {
        "x": rng.standard_normal((B, C, HH, WW), dtype=np.float32),
        "norm_w": np.ones(C, np.float32),
        "norm_b": np.zeros(C, np.float32),
        "wq": rng.standard_normal((C, C), dtype=np.float32) / np.sqrt(C),
        "bq": np.zeros(C, np.float32),
        "wk": rng.standard_normal((C, C), dtype=np.float32) / np.sqrt(C),
        "bk": np.zeros(C, np.float32),
        "wv": rng.standard_normal((C, C), dtype=np.float32) / np.sqrt(C),
        "bv": np.zeros(C, np.float32),
        "wp": rng.standard_normal((C, C), dtype=np.float32) / np.sqrt(C),
        "bp": np.zeros(C, np.float32),
    }
    out = kernel(**fake)
    print("kernel out", out.shape, out.dtype, float(np.abs(out).max()))
